# revision 14
# baseline (speedup 1.0000x reference)
"""MaxIoUAssigner on 8 Trainium2 NeuronCores (Bass/Tile) — v3.

kernel(bboxes[200000,4] f32, gt_bboxes[256,4] f32) -> assigned[200000] int32

Semantics (mmdet MaxIoUAssigner, pos=neg=0.5, min_pos_iou=0,
gt_max_assign_all=True):
  overlaps = iou(gt, priors)  [256, 200000]
  per-prior max/argmax (first index wins ties); <0.5 -> 0; >=0.5 -> argmax+1
  low-quality: priors attaining a gt's row max get gt_i+1 (later gt wins)

This target's cost model is dominated by a flat ~41us per *instruction*
(engine-independent, nearly size-independent); DMA bytes overlap compute.
The kernel is therefore architected to minimize instruction count:

 - Priors are sorted by x on the host and split into 8 contiguous x-bands
   (one per core, 25000 each); each band is split by y into two halves.
   Each half only interacts with the <=64 gts whose boxes can touch it
   (data-checked; host falls back to a numpy path if a cap is exceeded).
 - On-device layout: 128 partitions = 2 groups x 64 gts; free dim = the
   half's full 12500 priors in ONE chunk.  Coordinates roll through a
   single [128,2,12500] buffer (x-pair, then y-pair, then area), so the
   whole IoU core is 8 fat instructions on [128,12500] operands.
 - Per-prior argmax+max: iou is encoded as key = floor(iou*2^15) +
   (63-p)/64 (exact in f32: 16+6 bits) and max-reduced across partitions
   with gpsimd partition_all_reduce per group; the host decodes
   (bucketed argmax, exact 0.5 threshold via the floor bias).
 - Per-gt max+argmax (for the low-quality step) via one tensor_reduce
   into slot 0 of a -1e30-filled array + one max_index lookup (the top-8
   DVE `max` sort op costs ~4x a normal instruction and was replaced);
   candidates are combined on the host across halves/cores, removing the
   iou stash, the whole second phase, and the gt-max AllReduce collective
   of the v1 design.
 - 23 instructions per core per pass vs ~250 in v1.

Host does only O(N) label decode + argsort; all 51.2M-element IoU work
runs on device.
"""

import sys

if "/opt/trn_rl_repo" not in sys.path:
    sys.path.insert(0, "/opt/trn_rl_repo")

import numpy as np

from concourse import bacc, bass_utils, mybir, tile

f32 = mybir.dt.float32
i32 = mybir.dt.int32
u32 = mybir.dt.uint32
Alu = mybir.AluOpType
ActF = mybir.ActivationFunctionType

N_FULL = 200000
G = 256
P = 128
HG = 64                      # gts per group (2 groups of 64 partitions)
N_CORES = 8
NB = N_FULL // N_CORES       # 25000 priors per core (x-band)
NH = NB // 2                 # 12500 per y-half (one chunk)
F = NH
KSCALE = 32768.0             # 2^15 iou quantization for the key encode
KTHR = 16384                 # floor(iou*2^15) >= 16384  <=>  iou >= 0.5
DUMMY = 1.0e8                # far-away dummy gt coordinate


def build_program(repeat=1, n_cores=N_CORES):
    import concourse.bass_isa as bass_isa

    nc = bacc.Bacc("TRN2", target_bir_lowering=False, debug=False,
                   num_devices=n_cores)
    # rows per half: x1, x2, y1, y2, area
    bbx = nc.dram_tensor("bbx", [2, 5, NH], f32, kind="ExternalInput").ap()
    gtc_d = nc.dram_tensor("gtc", [P, 8], f32, kind="ExternalInput").ap()
    okey = nc.dram_tensor("okey", [2, NH], f32, kind="ExternalOutput").ap()
    ogvi = nc.dram_tensor("ogvi", [P, 16], u32, kind="ExternalOutput").ap()

    with tile.TileContext(nc) as tc:
        with (
            tc.tile_pool(name="c", bufs=1) as cpool,
            tc.tile_pool(name="w", bufs=1) as wpool,
        ):
            gtc = cpool.tile([P, 8], f32, tag="gtc")
            gvi = cpool.tile([P, 16], u32, tag="gvi")
            # slots 1-7 of the value block stay at -1e30 so max_index only
            # resolves slot 0 (the true row max from tensor_reduce)
            nc.gpsimd.memset(gvi.bitcast(f32)[:, 0:8], -1.0e30)

            nc.sync.dma_start(gtc[:], gtc_d)
            gx1, gx2 = gtc[:, 0:1], gtc[:, 1:2]
            gy1, gy2 = gtc[:, 2:3], gtc[:, 3:4]
            gar, frac = gtc[:, 4:5], gtc[:, 5:6]

            for _rep in range(repeat):
                pair = wpool.tile([P, 2, F], f32, tag="pair")  # 100 KB
                io = wpool.tile([P, F], f32, tag="io")         # 50 KB
                tmp = wpool.tile([P, F], f32, tag="tmp")       # 50 KB

                def ld(r0, r1, dst, dw):
                    # broadcast rows [r0:r1) of each half to its 64 parts
                    for h in range(2):
                        nc.sync.dma_start(
                            dst[h * HG:(h + 1) * HG, 0:dw],
                            bbx[h, r0:r1].rearrange("r n -> () r n")
                            .broadcast_to([HG, dw, F]))

                ld(0, 2, pair, 2)                              # x1, x2
                nc.vector.tensor_scalar(tmp[:], pair[:, 0], gx1, None,
                                        op0=Alu.max)
                nc.vector.scalar_tensor_tensor(io[:], pair[:, 1], gx2,
                                               tmp[:], op0=Alu.min,
                                               op1=Alu.subtract)
                ld(2, 4, pair, 2)                              # y1, y2
                nc.vector.tensor_scalar(tmp[:], pair[:, 0], gy1, None,
                                        op0=Alu.max)
                nc.vector.scalar_tensor_tensor(tmp[:], pair[:, 1], gy2,
                                               tmp[:], op0=Alu.min,
                                               op1=Alu.subtract)
                nc.vector.scalar_tensor_tensor(io[:], io[:], 0.0, tmp[:],
                                               op0=Alu.max, op1=Alu.mult)
                ld(4, 5, pair, 1)                              # area
                nc.vector.scalar_tensor_tensor(tmp[:], pair[:, 0], gar,
                                               io[:], op0=Alu.add,
                                               op1=Alu.subtract)
                nc.vector.reciprocal_approx_fast(tmp[:], tmp[:])
                nc.vector.tensor_mul(io[:], io[:], tmp[:])
                # per-gt row max + its index for the low-quality step
                nc.vector.tensor_reduce(gvi.bitcast(f32)[:, 0:1], io[:],
                                        axis=mybir.AxisListType.X,
                                        op=Alu.max)
                nc.vector.max_index(gvi[:, 8:16],
                                    gvi.bitcast(f32)[:, 0:8], io[:])
                # per-prior key encode: floor(iou*2^15) + (63-p)/64
                ki = wpool.tile([P, F], i32, tag="pair")  # reuse pair slot
                nc.scalar.activation(ki[:], io[:], ActF.Copy, bias=-0.5,
                                     scale=KSCALE)
                nc.scalar.activation(tmp[:], ki[:], ActF.Relu, bias=frac,
                                     scale=1.0)
                # group max across partitions (AR ucode is base-0 only:
                # copy group B down to partition 0; io's slot is free now)
                tmp2 = wpool.tile([HG, F], f32, tag="io")
                nc.sync.dma_start(tmp2[:], tmp[HG:P])
                nc.gpsimd.partition_all_reduce(
                    tmp[0:HG], tmp[0:HG], channels=HG,
                    reduce_op=bass_isa.ReduceOp.max)
                nc.gpsimd.partition_all_reduce(
                    tmp2[:], tmp2[:], channels=HG,
                    reduce_op=bass_isa.ReduceOp.max)
                nc.sync.dma_start(okey[0:1, :], tmp[0:1, :])
                nc.sync.dma_start(okey[1:2, :], tmp2[0:1, :])
            nc.sync.dma_start(ogvi, gvi[:])
    nc.compile()
    return nc


_NC_CACHE = None


def _get_program():
    global _NC_CACHE
    if _NC_CACHE is None:
        _NC_CACHE = build_program()
    return _NC_CACHE


def prepare_inputs(bb, gt):
    """Sort priors into 8 x-bands x 2 y-halves; pick each half's gts.

    Returns (in_maps, meta) where meta[k] = (halves_idx, gmaps):
    halves_idx[h] = global prior indices of half h (device column order),
    gmaps[h] = ascending global gt indices assigned to that half's group.
    Returns None if a gt group exceeds HG (caller falls back).
    """
    xorder = np.argsort(bb[:, 0], kind="stable")
    in_maps, meta = [], []
    for k in range(N_CORES):
        band_idx = xorder[k * NB:(k + 1) * NB]
        yord = np.argsort(bb[band_idx, 1], kind="stable")
        halves = [band_idx[yord[:NH]], band_idx[yord[NH:]]]
        bbx = np.empty((2, 5, NH), np.float32)
        gtc = np.zeros((P, 8), np.float32)
        gmaps = []
        for h in range(2):
            B = bb[halves[h]]
            bbx[h, 0] = B[:, 0]
            bbx[h, 1] = B[:, 2]
            bbx[h, 2] = B[:, 1]
            bbx[h, 3] = B[:, 3]
            bbx[h, 4] = (B[:, 2] - B[:, 0]) * (B[:, 3] - B[:, 1])
            sel = np.nonzero(
                (gt[:, 0] <= B[:, 2].max()) & (gt[:, 2] >= B[:, 0].min())
                & (gt[:, 1] <= B[:, 3].max()) & (gt[:, 3] >= B[:, 1].min())
            )[0]
            if len(sel) > HG:
                return None
            base = h * HG
            n = len(sel)
            gtc[base:base + n, 0] = gt[sel, 0]
            gtc[base:base + n, 1] = gt[sel, 2]
            gtc[base:base + n, 2] = gt[sel, 1]
            gtc[base:base + n, 3] = gt[sel, 3]
            gtc[base:base + n, 4] = ((gt[sel, 2] - gt[sel, 0])
                                     * (gt[sel, 3] - gt[sel, 1]))
            gtc[base + n:base + HG, 0] = DUMMY
            gtc[base + n:base + HG, 1] = DUMMY + 1.0
            gtc[base + n:base + HG, 2] = DUMMY
            gtc[base + n:base + HG, 3] = DUMMY + 1.0
            gtc[base + n:base + HG, 4] = 1.0
            gtc[base:base + HG, 5] = (HG - 1 - np.arange(HG)) / HG
            gmaps.append(sel)
        in_maps.append({"bbx": bbx, "gtc": gtc})
        meta.append((halves, gmaps))
    return in_maps, meta


def assemble(res, meta):
    """Decode per-prior keys + per-gt candidates into final labels."""
    assigned = np.zeros(N_FULL, np.int32)
    cand = [[] for _ in range(G)]  # per gt: list of (val, prior) candidates
    for k in range(N_CORES):
        halves, gmaps = meta[k]
        r = res.results[k]
        okey = r["okey"]                       # [2, NH]
        ogvi = r["ogvi"]                       # [P, 16] u32
        for h in range(2):
            v = okey[h].astype(np.float64)
            w = np.rint(v * HG).astype(np.int64)
            kib = w >> 6
            plocal = (HG - 1) - (w & (HG - 1))
            gsel = gmaps[h]
            gl = np.full(HG, -1, np.int64)
            gl[:len(gsel)] = gsel
            gwin = gl[np.clip(plocal, 0, HG - 1)]
            lab = np.where((kib >= KTHR) & (gwin >= 0), gwin + 1, 0)
            assigned[halves[h]] = lab
        # gt-side candidates
        val = ogvi[:, 0:8].view(np.float32)
        idx = ogvi[:, 8:16]
        for h in range(2):
            gsel = gmaps[h]
            if not len(gsel):
                continue
            base = h * HG
            pri = halves[h]
            for pl, g in enumerate(gsel):
                v0 = val[base + pl, 0]
                cand[g].append((v0, pri[idx[base + pl, 0]]))
                # exact ties within this half's top-8
                j = 1
                while j < 8 and val[base + pl, j] == v0:
                    cand[g].append((v0, pri[idx[base + pl, j]]))
                    j += 1
    for g in range(G):
        if not cand[g]:
            continue
        vmax = max(v for v, _ in cand[g])
        for v, p in cand[g]:
            if v == vmax:
                assigned[p] = g + 1
    return assigned


def _host_fallback(bb, gt):
    """Pure-numpy reference path (used only if a gt-group cap is hit)."""
    N = bb.shape[0]
    max_ov = np.zeros(N, np.float32)
    arg_ov = np.zeros(N, np.int64)
    gt_max = np.zeros(G, np.float32)
    area_g = (gt[:, 2] - gt[:, 0]) * (gt[:, 3] - gt[:, 1])
    area_b = (bb[:, 2] - bb[:, 0]) * (bb[:, 3] - bb[:, 1])
    step = 20000
    for s in range(0, N, step):
        e = min(s + step, N)
        lt = np.maximum(gt[:, None, :2], bb[None, s:e, :2])
        rb = np.minimum(gt[:, None, 2:], bb[None, s:e, 2:])
        wh = np.clip(rb - lt, 0, None).astype(np.float32)
        inter = wh[..., 0] * wh[..., 1]
        union = np.maximum(area_g[:, None] + area_b[None, s:e] - inter,
                           np.float32(1e-6))
        ov = inter / union
        max_ov[s:e] = ov.max(axis=0)
        arg_ov[s:e] = ov.argmax(axis=0)
        gt_max = np.maximum(gt_max, ov.max(axis=1))
        del lt, rb, wh, inter, union, ov
    assigned = np.where(max_ov >= 0.5, arg_ov + 1, 0).astype(np.int32)
    for s in range(0, N, step):
        e = min(s + step, N)
        lt = np.maximum(gt[:, None, :2], bb[None, s:e, :2])
        rb = np.minimum(gt[:, None, 2:], bb[None, s:e, 2:])
        wh = np.clip(rb - lt, 0, None).astype(np.float32)
        inter = wh[..., 0] * wh[..., 1]
        union = np.maximum(area_g[:, None] + area_b[None, s:e] - inter,
                           np.float32(1e-6))
        ov = inter / union
        eq = ov == gt_max[:, None]
        gidx = np.where(eq, np.arange(G)[:, None], -1).max(axis=0)
        sel = gidx >= 0
        assigned[s:e][sel] = gidx[sel] + 1
        del lt, rb, wh, inter, union, ov
    return assigned


def kernel(bboxes: np.ndarray, gt_bboxes: np.ndarray) -> np.ndarray:
    assert bboxes.shape == (N_FULL, 4) and gt_bboxes.shape == (G, 4)
    bb = np.ascontiguousarray(bboxes, dtype=np.float32)
    gt = np.ascontiguousarray(gt_bboxes, dtype=np.float32)
    prep = prepare_inputs(bb, gt)
    if prep is None:
        return _host_fallback(bb, gt)
    in_maps, meta = prep
    nc = _get_program()
    res = bass_utils.run_bass_kernel_spmd(nc, in_maps,
                                          core_ids=list(range(N_CORES)))
    return assemble(res, meta)


if __name__ == "__main__":
    rng = np.random.default_rng(0)
    bb_ = np.zeros((N_FULL, 4), np.float32)
    bb_[:, :2] = rng.uniform(0, 928, (N_FULL, 2))
    bb_[:, 2:] = bb_[:, :2] + rng.uniform(1, 97, (N_FULL, 2))
    gtb = np.zeros((G, 4), np.float32)
    gtb[:, :2] = rng.uniform(0, 928, (G, 2))
    gtb[:, 2:] = gtb[:, :2] + rng.uniform(1, 97, (G, 2))
    print(kernel(bb_, gtb)[:20])


# revision 15
# speedup vs baseline: 1.1579x; 1.1579x over previous
"""MaxIoUAssigner on 8 Trainium2 NeuronCores (Bass/Tile) — v3.

kernel(bboxes[200000,4] f32, gt_bboxes[256,4] f32) -> assigned[200000] int32

Semantics (mmdet MaxIoUAssigner, pos=neg=0.5, min_pos_iou=0,
gt_max_assign_all=True):
  overlaps = iou(gt, priors)  [256, 200000]
  per-prior max/argmax (first index wins ties); <0.5 -> 0; >=0.5 -> argmax+1
  low-quality: priors attaining a gt's row max get gt_i+1 (later gt wins)

This target's cost model is dominated by a flat ~41us per *instruction*
(engine-independent, nearly size-independent); DMA bytes overlap compute.
The kernel is therefore architected to minimize instruction count:

 - Priors are sorted by x on the host and split into 8 contiguous x-bands
   (one per core, 25000 each); each band is split by y into two halves.
   Each half only interacts with the <=64 gts whose boxes can touch it
   (data-checked; host falls back to a numpy path if a cap is exceeded).
 - On-device layout: 128 partitions = 2 groups x 64 gts; free dim = the
   half's full 12500 priors in ONE chunk.  Coordinates roll through a
   single [128,2,12500] buffer (x-pair, then y-pair, then area), so the
   whole IoU core is 8 fat instructions on [128,12500] operands.
 - Per-prior argmax+max: iou is encoded as key = floor(iou*2^15) +
   (63-p)/64 (exact in f32: 16+6 bits) and max-reduced across partitions
   with gpsimd partition_all_reduce per group; the host decodes
   (bucketed argmax, exact 0.5 threshold via the floor bias).
 - Per-gt max+argmax (for the low-quality step) via one tensor_reduce
   into slot 0 of a -1e30-filled array + one max_index lookup (the top-8
   DVE `max` sort op costs ~4x a normal instruction and was replaced);
   candidates are combined on the host across halves/cores, removing the
   iou stash, the whole second phase, and the gt-max AllReduce collective
   of the v1 design.
 - 23 instructions per core per pass vs ~250 in v1.

Host does only O(N) label decode + argsort; all 51.2M-element IoU work
runs on device.
"""

import sys

if "/opt/trn_rl_repo" not in sys.path:
    sys.path.insert(0, "/opt/trn_rl_repo")

import numpy as np

from concourse import bacc, bass_utils, mybir, tile

f32 = mybir.dt.float32
i32 = mybir.dt.int32
u32 = mybir.dt.uint32
Alu = mybir.AluOpType
ActF = mybir.ActivationFunctionType

N_FULL = 200000
G = 256
P = 128
HG = 64                      # gts per group (2 groups of 64 partitions)
N_CORES = 8
NB = N_FULL // N_CORES       # 25000 priors per core (x-band)
NH = NB // 2                 # 12500 per y-half (one chunk)
F = NH
KSCALE = 32768.0             # 2^15 iou quantization for the key encode
KTHR = 16384                 # floor(iou*2^15) >= 16384  <=>  iou >= 0.5
DUMMY = 1.0e8                # far-away dummy gt coordinate


def build_program(repeat=1, n_cores=N_CORES):
    import concourse.bass_isa as bass_isa

    nc = bacc.Bacc("TRN2", target_bir_lowering=False, debug=False,
                   num_devices=n_cores)
    # rows per half: x1, x2, y1, y2, area
    bbx = nc.dram_tensor("bbx", [2, 5, NH], f32, kind="ExternalInput").ap()
    gtc_d = nc.dram_tensor("gtc", [P, 8], f32, kind="ExternalInput").ap()
    okey = nc.dram_tensor("okey", [2, NH], f32, kind="ExternalOutput").ap()
    ogvi = nc.dram_tensor("ogvi", [P, 16], u32, kind="ExternalOutput").ap()

    with tile.TileContext(nc) as tc:
        with (
            tc.tile_pool(name="c", bufs=1) as cpool,
            tc.tile_pool(name="w", bufs=1) as wpool,
        ):
            gtc = cpool.tile([P, 8], f32, tag="gtc")
            gvi = cpool.tile([P, 16], u32, tag="gvi")
            # slots 1-7 of the value block stay at -1e30 so max_index only
            # resolves slot 0 (the true row max from tensor_reduce)
            nc.gpsimd.memset(gvi.bitcast(f32)[:, 0:8], -1.0e30)

            nc.sync.dma_start(gtc[:], gtc_d)
            gx1, gx2 = gtc[:, 0:1], gtc[:, 1:2]
            gy1, gy2 = gtc[:, 2:3], gtc[:, 3:4]
            gar, frac = gtc[:, 4:5], gtc[:, 5:6]

            for _rep in range(repeat):
                pair = wpool.tile([P, 2, F], f32, tag="pair")  # 100 KB
                io = wpool.tile([P, F], f32, tag="io")         # 50 KB
                tmp = wpool.tile([P, F], f32, tag="tmp")       # 50 KB

                def ld(r0, r1, dst, dw):
                    # broadcast rows [r0:r1) of each half to its 64 parts
                    for h in range(2):
                        nc.sync.dma_start(
                            dst[h * HG:(h + 1) * HG, 0:dw],
                            bbx[h, r0:r1].rearrange("r n -> () r n")
                            .broadcast_to([HG, dw, F]))

                ld(0, 2, pair, 2)                              # x1, x2
                nc.vector.tensor_scalar(tmp[:], pair[:, 0], gx1, None,
                                        op0=Alu.max)
                nc.vector.scalar_tensor_tensor(io[:], pair[:, 1], gx2,
                                               tmp[:], op0=Alu.min,
                                               op1=Alu.subtract)
                ld(2, 4, pair, 2)                              # y1, y2
                nc.vector.tensor_scalar(tmp[:], pair[:, 0], gy1, None,
                                        op0=Alu.max)
                nc.vector.scalar_tensor_tensor(tmp[:], pair[:, 1], gy2,
                                               tmp[:], op0=Alu.min,
                                               op1=Alu.subtract)
                nc.vector.scalar_tensor_tensor(io[:], io[:], 0.0, tmp[:],
                                               op0=Alu.max, op1=Alu.mult)
                ld(4, 5, pair, 1)                              # area
                nc.vector.scalar_tensor_tensor(tmp[:], pair[:, 0], gar,
                                               io[:], op0=Alu.add,
                                               op1=Alu.subtract)
                nc.vector.reciprocal(tmp[:], tmp[:])
                nc.vector.tensor_mul(io[:], io[:], tmp[:])
                # per-gt row max + its index for the low-quality step
                nc.vector.tensor_reduce(gvi.bitcast(f32)[:, 0:1], io[:],
                                        axis=mybir.AxisListType.X,
                                        op=Alu.max)
                nc.vector.max_index(gvi[:, 8:16],
                                    gvi.bitcast(f32)[:, 0:8], io[:])
                # per-prior key encode: floor(iou*2^15) + (63-p)/64
                ki = wpool.tile([P, F], i32, tag="pair")  # reuse pair slot
                nc.scalar.activation(ki[:], io[:], ActF.Copy, bias=-0.5,
                                     scale=KSCALE)
                nc.scalar.activation(tmp[:], ki[:], ActF.Relu, bias=frac,
                                     scale=1.0)
                # group max across partitions (AR ucode is base-0 only:
                # copy group B down to partition 0; io's slot is free now)
                tmp2 = wpool.tile([HG, F], f32, tag="io")
                nc.sync.dma_start(tmp2[:], tmp[HG:P])
                nc.gpsimd.partition_all_reduce(
                    tmp[0:HG], tmp[0:HG], channels=HG,
                    reduce_op=bass_isa.ReduceOp.max)
                nc.gpsimd.partition_all_reduce(
                    tmp2[:], tmp2[:], channels=HG,
                    reduce_op=bass_isa.ReduceOp.max)
                nc.sync.dma_start(okey[0:1, :], tmp[0:1, :])
                nc.sync.dma_start(okey[1:2, :], tmp2[0:1, :])
            nc.sync.dma_start(ogvi, gvi[:])
    nc.compile()
    return nc


_NC_CACHE = None


def _get_program():
    global _NC_CACHE
    if _NC_CACHE is None:
        _NC_CACHE = build_program()
    return _NC_CACHE


def prepare_inputs(bb, gt):
    """Sort priors into 8 x-bands x 2 y-halves; pick each half's gts.

    Returns (in_maps, meta) where meta[k] = (halves_idx, gmaps):
    halves_idx[h] = global prior indices of half h (device column order),
    gmaps[h] = ascending global gt indices assigned to that half's group.
    Returns None if a gt group exceeds HG (caller falls back).
    """
    xorder = np.argsort(bb[:, 0], kind="stable")
    in_maps, meta = [], []
    for k in range(N_CORES):
        band_idx = xorder[k * NB:(k + 1) * NB]
        yord = np.argsort(bb[band_idx, 1], kind="stable")
        halves = [band_idx[yord[:NH]], band_idx[yord[NH:]]]
        bbx = np.empty((2, 5, NH), np.float32)
        gtc = np.zeros((P, 8), np.float32)
        gmaps = []
        for h in range(2):
            B = bb[halves[h]]
            bbx[h, 0] = B[:, 0]
            bbx[h, 1] = B[:, 2]
            bbx[h, 2] = B[:, 1]
            bbx[h, 3] = B[:, 3]
            bbx[h, 4] = (B[:, 2] - B[:, 0]) * (B[:, 3] - B[:, 1])
            sel = np.nonzero(
                (gt[:, 0] <= B[:, 2].max()) & (gt[:, 2] >= B[:, 0].min())
                & (gt[:, 1] <= B[:, 3].max()) & (gt[:, 3] >= B[:, 1].min())
            )[0]
            if len(sel) > HG:
                return None
            base = h * HG
            n = len(sel)
            gtc[base:base + n, 0] = gt[sel, 0]
            gtc[base:base + n, 1] = gt[sel, 2]
            gtc[base:base + n, 2] = gt[sel, 1]
            gtc[base:base + n, 3] = gt[sel, 3]
            gtc[base:base + n, 4] = ((gt[sel, 2] - gt[sel, 0])
                                     * (gt[sel, 3] - gt[sel, 1]))
            gtc[base + n:base + HG, 0] = DUMMY
            gtc[base + n:base + HG, 1] = DUMMY + 1.0
            gtc[base + n:base + HG, 2] = DUMMY
            gtc[base + n:base + HG, 3] = DUMMY + 1.0
            gtc[base + n:base + HG, 4] = 1.0
            gtc[base:base + HG, 5] = (HG - 1 - np.arange(HG)) / HG
            gmaps.append(sel)
        in_maps.append({"bbx": bbx, "gtc": gtc})
        meta.append((halves, gmaps))
    return in_maps, meta


def assemble(res, meta):
    """Decode per-prior keys + per-gt candidates into final labels."""
    assigned = np.zeros(N_FULL, np.int32)
    cand = [[] for _ in range(G)]  # per gt: list of (val, prior) candidates
    for k in range(N_CORES):
        halves, gmaps = meta[k]
        r = res.results[k]
        okey = r["okey"]                       # [2, NH]
        ogvi = r["ogvi"]                       # [P, 16] u32
        for h in range(2):
            v = okey[h].astype(np.float64)
            w = np.rint(v * HG).astype(np.int64)
            kib = w >> 6
            plocal = (HG - 1) - (w & (HG - 1))
            gsel = gmaps[h]
            gl = np.full(HG, -1, np.int64)
            gl[:len(gsel)] = gsel
            gwin = gl[np.clip(plocal, 0, HG - 1)]
            lab = np.where((kib >= KTHR) & (gwin >= 0), gwin + 1, 0)
            assigned[halves[h]] = lab
        # gt-side candidates
        val = ogvi[:, 0:8].view(np.float32)
        idx = ogvi[:, 8:16]
        for h in range(2):
            gsel = gmaps[h]
            if not len(gsel):
                continue
            base = h * HG
            pri = halves[h]
            for pl, g in enumerate(gsel):
                v0 = val[base + pl, 0]
                cand[g].append((v0, pri[idx[base + pl, 0]]))
                # exact ties within this half's top-8
                j = 1
                while j < 8 and val[base + pl, j] == v0:
                    cand[g].append((v0, pri[idx[base + pl, j]]))
                    j += 1
    for g in range(G):
        if not cand[g]:
            continue
        vmax = max(v for v, _ in cand[g])
        for v, p in cand[g]:
            if v == vmax:
                assigned[p] = g + 1
    return assigned


def _host_fallback(bb, gt):
    """Pure-numpy reference path (used only if a gt-group cap is hit)."""
    N = bb.shape[0]
    max_ov = np.zeros(N, np.float32)
    arg_ov = np.zeros(N, np.int64)
    gt_max = np.zeros(G, np.float32)
    area_g = (gt[:, 2] - gt[:, 0]) * (gt[:, 3] - gt[:, 1])
    area_b = (bb[:, 2] - bb[:, 0]) * (bb[:, 3] - bb[:, 1])
    step = 20000
    for s in range(0, N, step):
        e = min(s + step, N)
        lt = np.maximum(gt[:, None, :2], bb[None, s:e, :2])
        rb = np.minimum(gt[:, None, 2:], bb[None, s:e, 2:])
        wh = np.clip(rb - lt, 0, None).astype(np.float32)
        inter = wh[..., 0] * wh[..., 1]
        union = np.maximum(area_g[:, None] + area_b[None, s:e] - inter,
                           np.float32(1e-6))
        ov = inter / union
        max_ov[s:e] = ov.max(axis=0)
        arg_ov[s:e] = ov.argmax(axis=0)
        gt_max = np.maximum(gt_max, ov.max(axis=1))
        del lt, rb, wh, inter, union, ov
    assigned = np.where(max_ov >= 0.5, arg_ov + 1, 0).astype(np.int32)
    for s in range(0, N, step):
        e = min(s + step, N)
        lt = np.maximum(gt[:, None, :2], bb[None, s:e, :2])
        rb = np.minimum(gt[:, None, 2:], bb[None, s:e, 2:])
        wh = np.clip(rb - lt, 0, None).astype(np.float32)
        inter = wh[..., 0] * wh[..., 1]
        union = np.maximum(area_g[:, None] + area_b[None, s:e] - inter,
                           np.float32(1e-6))
        ov = inter / union
        eq = ov == gt_max[:, None]
        gidx = np.where(eq, np.arange(G)[:, None], -1).max(axis=0)
        sel = gidx >= 0
        assigned[s:e][sel] = gidx[sel] + 1
        del lt, rb, wh, inter, union, ov
    return assigned


def kernel(bboxes: np.ndarray, gt_bboxes: np.ndarray) -> np.ndarray:
    assert bboxes.shape == (N_FULL, 4) and gt_bboxes.shape == (G, 4)
    bb = np.ascontiguousarray(bboxes, dtype=np.float32)
    gt = np.ascontiguousarray(gt_bboxes, dtype=np.float32)
    prep = prepare_inputs(bb, gt)
    if prep is None:
        return _host_fallback(bb, gt)
    in_maps, meta = prep
    nc = _get_program()
    res = bass_utils.run_bass_kernel_spmd(nc, in_maps,
                                          core_ids=list(range(N_CORES)))
    return assemble(res, meta)


if __name__ == "__main__":
    rng = np.random.default_rng(0)
    bb_ = np.zeros((N_FULL, 4), np.float32)
    bb_[:, :2] = rng.uniform(0, 928, (N_FULL, 2))
    bb_[:, 2:] = bb_[:, :2] + rng.uniform(1, 97, (N_FULL, 2))
    gtb = np.zeros((G, 4), np.float32)
    gtb[:, :2] = rng.uniform(0, 928, (G, 2))
    gtb[:, 2:] = gtb[:, :2] + rng.uniform(1, 97, (G, 2))
    print(kernel(bb_, gtb)[:20])
